# revision 4
# baseline (speedup 1.0000x reference)
"""RBF kernel exp(-gamma * ||x - c||^2) on 8 TRN2 NeuronCores.

Problem: x [4096, 2048] fp32, centers [4096, 2048] fp32, gamma = 0.05,
out [4096, 4096] fp32 = exp(-gamma * (||x||^2 + ||c||^2 - 2 x @ c.T)).

Strategy (hardcoded):
  - 2D shard over a 4 (batch) x 2 (centers) core grid: each core computes a
    [1024, 2048] output block from x rows [1024, 2048] and center rows
    [2048, 2048]; operands are fully SBUF-resident.
  - Host-side layout prep as part of sharding: operands are passed K-major
    (transposed), quantized to fp8-e4m3 for the cross-term matmuls, and laid
    out partition-major so every DMA chunk is contiguous per partition (2 KB
    runs -> fat SDMA descriptors). The squared-norm vectors are computed on
    host in fp32 (O(N*D), 0.1% of the GEMM FLOPs) and folded into the
    on-device epilogue.
  - On device: 256 fp8 DoubleRow matmuls (M=128, N=512, K=256 each)
    accumulate cross = x @ c.T into PSUM at 2 MACs/cell/cycle; DVE computes
    2*gamma*cross - gamma*||c||^2 from PSUM; ACT applies
    exp(. - gamma*||x||^2) with a per-partition bias and writes bf16
    (host upcasts; well inside tolerance); per-mi row DMAs ship the output,
    with the final mi split into per-ni pieces so the tail drain is short.
  - All input DMAs ride the SP (sync) HWDGE ring as its first instructions,
    interleaved in the exact order the opening pass consumes them, so the
    ACT-table load (hoisted to the scalar ring's head) no longer delays
    operand loading. The first k-chunk is split in half so the first real
    matmul starts ~0.5us earlier.
  - PE warm-up: a handful of dummy matmuls on a gpsimd-memset tile issued
    right after the engine preamble keep the PE busy from engine-start so
    HAM un-throttles to 2.4 GHz with minimal cold time on real matmuls;
    the opening pass runs k-outer across all 8 PSUM banks so the PE chews
    newly-arrived chunks immediately.
"""

import numpy as np
import ml_dtypes

import concourse.bass as bass
from concourse import bacc
import concourse.tile as tile
import concourse.mybir as mybir
from concourse import bass_utils

P = 128
B, C, D = 4096, 4096, 2048
GAMMA = 0.05

# core grid: 4 batch shards x 2 center shards
GB, GC = 4, 2
MB = B // GB  # 1024 rows of x per core
NB = C // GC  # 2048 center rows per core

KT = D // P  # 16 k-tiles
KP = KT // 2  # 8 DoubleRow k-pairs
MT = MB // P  # 8 m-tiles
NFREE = 512
NT = NB // NFREE  # 4 n-tiles
KH = KP // 2  # 4 ct kp-pair chunks per n-tile

FP8 = mybir.dt.float8e4
BF16 = mybir.dt.bfloat16
NWARM = 4  # dummy PE warm-up matmuls bridging engine-start -> first operands


def _build():
    nc = bacc.Bacc("TRN2", target_bir_lowering=False, debug=False, num_devices=8)
    # partition-major layouts: each chunk is contiguous per partition in HBM
    xt0 = nc.dram_tensor("xt0", [2, P, 2, MB // 2], FP8, kind="ExternalInput")
    xt = nc.dram_tensor("xt", [KP - 1, P, 2, MB], FP8, kind="ExternalInput")
    ct = nc.dram_tensor("ct", [NT, KH, P, 2, 2, NFREE], FP8, kind="ExternalInput")
    c2row = nc.dram_tensor("c2row", [1, NB], mybir.dt.float32, kind="ExternalInput")
    nx2 = nc.dram_tensor("nx2", [P, MT], mybir.dt.float32, kind="ExternalInput")
    out = nc.dram_tensor("out", [P, MT, NB], BF16, kind="ExternalOutput")

    with tile.TileContext(nc) as tc:
        with (
            tc.tile_pool(name="inp", bufs=1) as inp,
            tc.tile_pool(name="psum", bufs=8, space="PSUM") as psum_pool,
            tc.tile_pool(name="work", bufs=6) as work,
        ):
            c2g_sb = inp.tile([P, NB], mybir.dt.float32, tag="c2g")
            c2r_sb = inp.tile([1, NB], mybir.dt.float32, tag="c2r")
            nx2_sb = inp.tile([P, MT], mybir.dt.float32, tag="nx2")
            stage = inp.tile([P, MT, NB], BF16, tag="stage")
            zwarm = inp.tile([P, NFREE], FP8, tag="zwarm")

            # gpsimd (earliest-starting engine): unblock PE warm-up first,
            # then the tiny epilogue vectors via SWDGE + partition broadcast
            nc.gpsimd.memset(zwarm[:], 0)
            nc.gpsimd.dma_start(nx2_sb[:], nx2.ap())
            nc.gpsimd.dma_start(c2r_sb[:], c2row.ap())
            nc.gpsimd.partition_broadcast(c2g_sb[:], c2r_sb[:])

            # operand tiles
            xt_sb = [
                inp.tile([P, 2, MB], FP8, name=f"xt{kp}", tag=f"xt{kp}")
                for kp in range(KP)
            ]
            # ct_sb[ni][kh] covers kp = 2*kh, 2*kh+1 -> [P, kpi, 2, NFREE]
            ct_sb = [
                [
                    inp.tile(
                        [P, 2, 2, NFREE], FP8, name=f"ct{ni}_{kh}", tag=f"ct{ni}_{kh}"
                    )
                    for kh in range(KH)
                ]
                for ni in range(NT)
            ]

            # all operand loads on the SP ring, in consumption order; the
            # first k-chunk is split so the opening matmul starts sooner
            nc.sync.dma_start(ct_sb[0][0][:], ct.ap()[0, 0])
            nc.sync.dma_start(xt_sb[0][:, :, : MB // 2], xt0.ap()[0])
            nc.sync.dma_start(xt_sb[0][:, :, MB // 2 :], xt0.ap()[1])
            nc.sync.dma_start(xt_sb[1][:], xt.ap()[0])
            for kh in range(1, KH):
                nc.sync.dma_start(ct_sb[0][kh][:], ct.ap()[0, kh])
                nc.sync.dma_start(xt_sb[2 * kh][:], xt.ap()[2 * kh - 1])
                nc.sync.dma_start(xt_sb[2 * kh + 1][:], xt.ap()[2 * kh])
            for ni in range(1, NT):
                for kh in range(KH):
                    nc.sync.dma_start(ct_sb[ni][kh][:], ct.ap()[ni, kh])

            def epilogue(ps, mi, ni):
                t = work.tile([P, NFREE], mybir.dt.float32, tag="t")
                # t = 2*gamma*cross - gamma*||c||^2
                nc.vector.scalar_tensor_tensor(
                    t[:],
                    ps[:],
                    2.0 * GAMMA,
                    c2g_sb[:, bass.ts(ni, NFREE)],
                    mybir.AluOpType.mult,
                    mybir.AluOpType.subtract,
                )
                # stage[:, mi, ni*512:...] = exp(t - gamma*||x||^2) in bf16
                nc.scalar.activation(
                    stage[:, mi, bass.ts(ni, NFREE)],
                    t[:],
                    mybir.ActivationFunctionType.Exp,
                    bias=nx2_sb[:, mi : mi + 1],
                    scale=1.0,
                )

            def matmul(ps, mi, ni, kp):
                nc.tensor.matmul(
                    ps[:],
                    xt_sb[kp][:, :, bass.ts(mi, P)],
                    ct_sb[ni][kp // 2][:, kp % 2],
                    start=(kp == 0),
                    stop=(kp == KP - 1),
                    perf_mode=mybir.MatmulPerfMode.DoubleRow,
                )

            # PE warm-up: dummy matmuls on the memset tile bridge the gap
            # between engine-start and first-operand arrival so HAM ramps
            ps0 = [
                psum_pool.tile([P, NFREE], mybir.dt.float32, name=f"ps0_{mi}", tag="ps")
                for mi in range(MT)
            ]
            for _ in range(NWARM):
                nc.tensor.matmul(
                    ps0[0][:],
                    zwarm[:, :P],
                    zwarm[:],
                    start=True,
                    stop=True,
                    skip_group_check=True,
                )

            # opening pass (ni=0): k-outer across all 8 psum banks -> PE
            # consumes each newly-arrived k-chunk across all 8 m-tiles
            for kp in range(KP):
                for mi in range(MT):
                    matmul(ps0[mi], mi, 0, kp)
            for mi in range(MT):
                epilogue(ps0[mi], mi, 0)
            # ship the last mi's ni=0 piece early; its row DMA at the end
            # would otherwise lengthen the tail
            nc.scalar.dma_start(
                out.ap()[:, MT - 1, bass.ts(0, NFREE)],
                stage[:, MT - 1, bass.ts(0, NFREE)],
            )

            # main passes: m-outer, k-outer/n-inner (stationary reused 3x)
            for mi in range(MT):
                pss = [
                    psum_pool.tile(
                        [P, NFREE], mybir.dt.float32, name=f"ps_{mi}_{j}", tag="ps"
                    )
                    for j in range(NT - 1)
                ]
                for kp in range(KP):
                    for ni in range(1, NT):
                        matmul(pss[ni - 1], mi, ni, kp)
                for ni in range(1, NT):
                    epilogue(pss[ni - 1], mi, ni)
                if mi < MT - 1:
                    # whole-row output DMA [P, 2048] bf16 (4 KB/partition)
                    nc.scalar.dma_start(out.ap()[:, mi], stage[:, mi])
                else:
                    # final mi: per-ni pieces so the last transfer is small
                    for ni in range(1, NT):
                        nc.scalar.dma_start(
                            out.ap()[:, mi, bass.ts(ni, NFREE)],
                            stage[:, mi, bass.ts(ni, NFREE)],
                        )
    nc.finalize()
    return nc


def kernel(x: np.ndarray, centers: np.ndarray) -> np.ndarray:
    x = np.asarray(x, dtype=np.float32)
    centers = np.asarray(centers, dtype=np.float32)
    assert x.shape == (B, D) and centers.shape == (C, D)

    # host-side shard + layout prep
    np_fp8 = mybir.dt.np(FP8)
    x2 = GAMMA * (x.astype(np.float64) ** 2).sum(1).astype(np.float32)  # [B]
    c2 = GAMMA * (centers.astype(np.float64) ** 2).sum(1).astype(np.float32)  # [C]
    # K-major fp8: [D, B] -> [KP, 2, P, M] per-chunk partition-major
    xt_full = np.ascontiguousarray(x.T).astype(np_fp8)  # [D, B]
    ct_full = np.ascontiguousarray(centers.T).astype(np_fp8)  # [D, C]

    in_maps = []
    for core in range(8):
        bi, cj = divmod(core, GC)
        xs = xt_full[:, bi * MB : (bi + 1) * MB]  # [D, MB]
        cs = ct_full[:, cj * NB : (cj + 1) * NB]  # [D, NB]
        # xs [D, MB] -> [KP, 2, P, MB] -> chunk kp: [P, 2, MB]
        xr = xs.reshape(KP, 2, P, MB).transpose(0, 2, 1, 3)  # [KP, P, 2, MB]
        xt0_a = np.ascontiguousarray(
            xr[0].reshape(P, 2, 2, MB // 2).transpose(2, 0, 1, 3)
        )  # [2, P, 2, MB//2]
        xt_a = np.ascontiguousarray(xr[1:])  # [KP-1, P, 2, MB]
        # cs [D, NB] -> [NT, KH, P, 2(kpi), 2(ko), NFREE]
        cr = cs.reshape(KH, 2, 2, P, NT, NFREE)  # [kh, kpi, ko, P, ni, n]
        ct_a = np.ascontiguousarray(cr.transpose(4, 0, 3, 1, 2, 5))
        c2row = np.ascontiguousarray(c2[None, cj * NB : (cj + 1) * NB])
        nx2 = np.ascontiguousarray((-x2[bi * MB : (bi + 1) * MB]).reshape(MT, P).T)
        in_maps.append(
            {"xt0": xt0_a, "xt": xt_a, "ct": ct_a, "c2row": c2row, "nx2": nx2}
        )

    nc = _build()
    res = bass_utils.run_bass_kernel_spmd(nc, in_maps, core_ids=list(range(8)))

    out = np.empty((B, C), dtype=np.float32)
    for core in range(8):
        bi, cj = divmod(core, GC)
        blk = np.asarray(res.results[core]["out"])  # [P, MT, NB] bf16
        out[bi * MB : (bi + 1) * MB, cj * NB : (cj + 1) * NB] = (
            blk.transpose(1, 0, 2).reshape(MB, NB).astype(np.float32)
        )
    return out


# revision 5
# speedup vs baseline: 1.0414x; 1.0414x over previous
"""RBF kernel exp(-gamma * ||x - c||^2) on 8 TRN2 NeuronCores.

Problem: x [4096, 2048] fp32, centers [4096, 2048] fp32, gamma = 0.05,
out [4096, 4096] fp32 = exp(-gamma * (||x||^2 + ||c||^2 - 2 x @ c.T)).

Strategy (hardcoded):
  - 2D shard over a 4 (batch) x 2 (centers) core grid: each core computes a
    [1024, 2048] output block from x rows [1024, 2048] and center rows
    [2048, 2048]; operands are fully SBUF-resident.
  - Host-side layout prep as part of sharding: operands are passed K-major
    (transposed), quantized to fp8-e4m3 for the cross-term matmuls, and laid
    out partition-major so every DMA chunk is contiguous per partition.
    The squared-norm vectors are computed on host in fp32 and folded into
    the on-device epilogue; gamma*||c||^2 is pre-broadcast across partitions
    on host so no on-device broadcast is needed.
  - On device: 256 fp8 DoubleRow matmuls (M=128, N=512, K=256 each)
    accumulate cross = x @ c.T into PSUM at 2 MACs/cell/cycle; DVE computes
    2*gamma*cross - gamma*||c||^2 from PSUM; ACT applies
    exp(. - gamma*||x||^2) with a per-partition bias, writing bf16 into a
    per-n-tile staging buffer (host upcasts; well inside tolerance).
  - HWDGE dma_starts cost ~650ns of issuing-sequencer time each, so DMAs
    are few and fat: input in 17 coarse-to-fine chunks (small first chunks
    so the opening matmul starts early, 1 MB chunks later), output in 6
    per-n-tile row DMAs with the final n-tile split so the tail transfer is
    small. All data DMAs ride the SP ring; the ACT ring only runs the
    activation table load + exp epilogues.
  - PE warm-up: dummy matmuls on a gpsimd-memset tile right after the
    engine preamble keep HAM ramping while the first operands stream in;
    the opening pass (ni=0) runs k-outer across all 8 PSUM banks so the PE
    consumes each newly-arrived k-chunk immediately; the main passes run
    n-outer so epilogues stay evenly spread and PSUM pressure is low.
  - A tiny leading DVE memset pulls the Vector engine's instruction fetch
    ahead of the DMA jam (otherwise its first epilogue slips by ~5us).
"""

import numpy as np
import ml_dtypes

import concourse.bass as bass
from concourse import bacc
import concourse.tile as tile
import concourse.mybir as mybir
from concourse import bass_utils

P = 128
B, C, D = 4096, 4096, 2048
GAMMA = 0.05

# core grid: 4 batch shards x 2 center shards
GB, GC = 4, 2
MB = B // GB  # 1024 rows of x per core
NB = C // GC  # 2048 center rows per core

KT = D // P  # 16 k-tiles
KP = KT // 2  # 8 DoubleRow k-pairs
MT = MB // P  # 8 m-tiles
NFREE = 512
NT = NB // NFREE  # 4 n-tiles

FP8 = mybir.dt.float8e4
BF16 = mybir.dt.bfloat16
NWARM = 4  # dummy PE warm-up matmuls bridging engine-start -> first operands

# coarse-to-fine kp chunking for the opening operands (xt / ct ni=0)
KP_CHUNKS = [(0, 1), (1, 2), (2, 4), (4, 6), (6, 8)]


def _build():
    nc = bacc.Bacc("TRN2", target_bir_lowering=False, debug=False, num_devices=8)
    # partition-major layouts: chunks are contiguous per partition in HBM
    xt = nc.dram_tensor("xt", [P, KP, 2, MB], FP8, kind="ExternalInput")
    ct = nc.dram_tensor("ct", [NT, P, KP, 2, NFREE], FP8, kind="ExternalInput")
    c2g = nc.dram_tensor("c2g", [NT, P, NFREE], mybir.dt.float32, kind="ExternalInput")
    nx2 = nc.dram_tensor("nx2", [P, MT], mybir.dt.float32, kind="ExternalInput")
    out = nc.dram_tensor("out", [NT, P, MT, NFREE], BF16, kind="ExternalOutput")

    with tile.TileContext(nc) as tc:
        with (
            tc.tile_pool(name="inp", bufs=1) as inp,
            tc.tile_pool(name="psum", bufs=8, space="PSUM") as psum_pool,
            tc.tile_pool(name="work", bufs=6) as work,
        ):
            c2g_sb = inp.tile([P, NB], mybir.dt.float32, tag="c2g")
            nx2_sb = inp.tile([P, MT], mybir.dt.float32, tag="nx2")
            zwarm = inp.tile([P, NFREE], FP8, tag="zwarm")
            vboot = work.tile([P, 4], mybir.dt.float32, tag="vboot")
            stage = [
                inp.tile([P, MT, NFREE], BF16, name=f"stage{ni}", tag=f"stage{ni}")
                for ni in range(NT)
            ]
            xt_sb = inp.tile([P, KP, 2, MB], FP8, tag="xt")
            ct_sb = [
                inp.tile([P, KP, 2, NFREE], FP8, name=f"ct{ni}", tag=f"ct{ni}")
                for ni in range(NT)
            ]

            # vector: a tiny first instruction pulls this engine's
            # instruction stream ahead of the input-DMA jam
            nc.vector.memset(vboot[:], 0)
            # gpsimd: unblock PE warm-up, then the tiny bias vector (SWDGE)
            nc.gpsimd.memset(zwarm[:], 0)
            nc.gpsimd.dma_start(nx2_sb[:], nx2.ap())

            # operand loads on the SP ring in consumption order,
            # coarse-to-fine: small leading chunks, 1 MB steady-state
            for lo, hi in KP_CHUNKS:
                nc.sync.dma_start(xt_sb[:, lo:hi], xt.ap()[:, lo:hi])
                nc.sync.dma_start(ct_sb[0][:, lo:hi], ct.ap()[0, :, lo:hi])
            nc.sync.dma_start(
                c2g_sb[:, bass.ts(0, NFREE)], c2g.ap()[0]
            )
            nc.sync.dma_start(c2g_sb[:, bass.ts(1, NFREE)], c2g.ap()[1])
            nc.sync.dma_start(ct_sb[1][:], ct.ap()[1])
            nc.sync.dma_start(c2g_sb[:, bass.ts(2, NFREE)], c2g.ap()[2])
            nc.sync.dma_start(ct_sb[2][:], ct.ap()[2])
            nc.sync.dma_start(c2g_sb[:, bass.ts(3, NFREE)], c2g.ap()[3])
            nc.sync.dma_start(ct_sb[3][:], ct.ap()[3])

            def epilogue(ps, mi, ni):
                t = work.tile([P, NFREE], mybir.dt.float32, tag="t")
                # t = 2*gamma*cross - gamma*||c||^2
                nc.vector.scalar_tensor_tensor(
                    t[:],
                    ps[:],
                    2.0 * GAMMA,
                    c2g_sb[:, bass.ts(ni, NFREE)],
                    mybir.AluOpType.mult,
                    mybir.AluOpType.subtract,
                )
                # stage[ni][:, mi, :] = exp(t - gamma*||x||^2) in bf16
                nc.scalar.activation(
                    stage[ni][:, mi],
                    t[:],
                    mybir.ActivationFunctionType.Exp,
                    bias=nx2_sb[:, mi : mi + 1],
                    scale=1.0,
                )

            def matmul(ps, mi, ni, kp):
                nc.tensor.matmul(
                    ps[:],
                    xt_sb[:, kp, :, bass.ts(mi, P)],
                    ct_sb[ni][:, kp],
                    start=(kp == 0),
                    stop=(kp == KP - 1),
                    perf_mode=mybir.MatmulPerfMode.DoubleRow,
                )

            # PE warm-up while the first operand chunks stream in
            ps0 = [
                psum_pool.tile([P, NFREE], mybir.dt.float32, name=f"ps0_{mi}", tag="ps")
                for mi in range(MT)
            ]
            for _ in range(NWARM):
                nc.tensor.matmul(
                    ps0[0][:],
                    zwarm[:, :P],
                    zwarm[:],
                    start=True,
                    stop=True,
                    skip_group_check=True,
                )

            # opening pass (ni=0): k-outer across all 8 psum banks -> PE
            # consumes each newly-arrived k-chunk across all 8 m-tiles
            for kp in range(KP):
                for mi in range(MT):
                    matmul(ps0[mi], mi, 0, kp)
            for mi in range(MT):
                epilogue(ps0[mi], mi, 0)
            nc.sync.dma_start(out.ap()[0], stage[0][:])

            # main passes: n-outer, mi-middle, k-inner; epilogues stay
            # evenly spread and at most 2-3 psum banks are in flight
            for ni in range(1, NT):
                for mi in range(MT):
                    ps = psum_pool.tile(
                        [P, NFREE], mybir.dt.float32, name=f"ps_{ni}_{mi}", tag="ps"
                    )
                    for kp in range(KP):
                        matmul(ps, mi, ni, kp)
                    epilogue(ps, mi, ni)
                if ni < NT - 1:
                    nc.sync.dma_start(out.ap()[ni], stage[ni][:])
                else:
                    # final n-tile: split so the tail transfer is small
                    nc.sync.dma_start(
                        out.ap()[ni, :, 0 : MT - 2], stage[ni][:, 0 : MT - 2]
                    )
                    nc.sync.dma_start(
                        out.ap()[ni, :, MT - 2], stage[ni][:, MT - 2]
                    )
                    nc.sync.dma_start(
                        out.ap()[ni, :, MT - 1], stage[ni][:, MT - 1]
                    )
    nc.finalize()
    return nc


def kernel(x: np.ndarray, centers: np.ndarray) -> np.ndarray:
    x = np.asarray(x, dtype=np.float32)
    centers = np.asarray(centers, dtype=np.float32)
    assert x.shape == (B, D) and centers.shape == (C, D)

    # host-side shard + layout prep
    np_fp8 = mybir.dt.np(FP8)
    x2 = GAMMA * (x.astype(np.float64) ** 2).sum(1).astype(np.float32)  # [B]
    c2 = GAMMA * (centers.astype(np.float64) ** 2).sum(1).astype(np.float32)  # [C]
    xt_full = np.ascontiguousarray(x.T).astype(np_fp8)  # [D, B]
    ct_full = np.ascontiguousarray(centers.T).astype(np_fp8)  # [D, C]

    in_maps = []
    for core in range(8):
        bi, cj = divmod(core, GC)
        xs = xt_full[:, bi * MB : (bi + 1) * MB]  # [D, MB]
        cs = ct_full[:, cj * NB : (cj + 1) * NB]  # [D, NB]
        # [D, MB] -> [P, KP, 2, MB]
        xt_a = np.ascontiguousarray(xs.reshape(KP, 2, P, MB).transpose(2, 0, 1, 3))
        # [D, NB] -> [NT, P, KP, 2, NFREE]
        ct_a = np.ascontiguousarray(
            cs.reshape(KP, 2, P, NT, NFREE).transpose(3, 2, 0, 1, 4)
        )
        c2s = c2[cj * NB : (cj + 1) * NB].reshape(NT, 1, NFREE)
        c2g_a = np.ascontiguousarray(np.broadcast_to(c2s, (NT, P, NFREE)))
        nx2_a = np.ascontiguousarray((-x2[bi * MB : (bi + 1) * MB]).reshape(MT, P).T)
        in_maps.append({"xt": xt_a, "ct": ct_a, "c2g": c2g_a, "nx2": nx2_a})

    nc = _build()
    res = bass_utils.run_bass_kernel_spmd(nc, in_maps, core_ids=list(range(8)))

    out = np.empty((B, C), dtype=np.float32)
    for core in range(8):
        bi, cj = divmod(core, GC)
        blk = np.asarray(res.results[core]["out"])  # [NT, P, MT, NFREE] bf16
        out[bi * MB : (bi + 1) * MB, cj * NB : (cj + 1) * NB] = (
            blk.transpose(2, 1, 0, 3).reshape(MB, NB).astype(np.float32)
        )
    return out
